# revision 11
# baseline (speedup 1.0000x reference)
"""DeepSpeed-style self-attention block (RMSNorm + QKV + RoPE + causal attention
+ output projection) on 8 Trainium2 NeuronCores.

Sharding: tensor-parallel over heads (16 heads -> 2 per core). Each core computes
its 2 heads' attention over the full sequence and a partial output projection over
its 256-dim slice of the context; the 8 partial outputs are summed on the host
(the TP all-reduce equivalent, done at gather time).

All PE matmuls run in bfloat16 (1 cyc/row at any ap size). Per-token RMS stats and
softmax denominators are computed with ap=1 matmuls (ones moving tensor, data as
stationary), which cost ~nothing on the PE. V is projected directly into natural
[token, dv] layout (x stationary, weights moving) so no PE transposes or extra
copies are needed; the RMS scale s is applied at V eviction via the activation
engine's per-partition scale operand. rsqrt is computed as exp(-0.5*ln(x)).
Scores/pv/exp are trimmed to the causal region at 128-column granularity, and the
causal mask shrinks to a single 128x128 triangle tile applied only on the exact
diagonal blocks.

Schedule (software pipelined, one iteration per chunk i):
  PE:  oproj(i-1) | passA(i+1) [q-phase, k-phase, v-phase, stats] | attn(i)
The RMS->RoPE chain of chunk i+1 executes on Act/Pool/DVE underneath oproj(i)/
passA(i+2), so the PE never waits for q/k eviction. DMA is batched (2 loads +
4 quarter-stores per 512-token chunk); stores issue from the activation engine's
queue right after their evictions.
"""
import sys
sys.path.insert(0, '/opt/trn_rl_repo')

import math
import numpy as np
from contextlib import ExitStack

import concourse.bass as bass
from concourse import bacc
import concourse.mybir as mybir
import concourse.tile as tile
from concourse import bass_utils
from concourse.masks import make_identity

# ---- problem constants (hardcoded per contest contract) ----
B, S, H, HEADS, D = 2, 2048, 2048, 16, 128
NT = B * S                    # 4096 tokens
NCORES = 8
HPC = HEADS // NCORES         # 2 heads per core
OC = HPC * D                  # 256 output dims per core
P = 128
CH = 512                      # token chunk
NCH = NT // CH                # 8 chunks
KT = H // P                   # 16 d-tiles
CPB = S // CH                 # 4 chunks per batch
HD = D // 2
SCALE = 1.0 / math.sqrt(D)
RMS_EPS = 1e-6
ROPE_BASE = 10000.0
MASK_VAL = -10000.0

F32 = mybir.dt.float32
BF16 = mybir.dt.bfloat16
EXP = mybir.ActivationFunctionType.Exp
LN = mybir.ActivationFunctionType.Ln
COPY = mybir.ActivationFunctionType.Copy


def build_module():
    nc = bacc.Bacc("TRN2", target_bir_lowering=False, debug=False, num_devices=NCORES)

    xT = nc.dram_tensor("xT", [H, NT], BF16, kind="ExternalInput").ap()
    wqT = nc.dram_tensor("wqT", [H, OC], BF16, kind="ExternalInput").ap()
    wkT = nc.dram_tensor("wkT", [H, OC], BF16, kind="ExternalInput").ap()
    wvT = nc.dram_tensor("wvT", [H, OC], BF16, kind="ExternalInput").ap()
    woT = nc.dram_tensor("woT", [OC, H], BF16, kind="ExternalInput").ap()
    trigT = nc.dram_tensor("trigT", [2, D, NT], BF16, kind="ExternalInput").ap()
    maskT = nc.dram_tensor("maskT", [P, P], BF16, kind="ExternalInput").ap()
    onesT = nc.dram_tensor("onesT", [P, 1], BF16, kind="ExternalInput").ap()
    identT = nc.dram_tensor("identT", [P, P], BF16, kind="ExternalInput").ap()
    out_p = nc.dram_tensor("out_p", [NT, H], BF16, kind="ExternalOutput").ap()

    xTr = xT.rearrange("(t p) n -> p t n", p=P)
    trig_r = trigT.rearrange("s d n -> d s n")
    out_pr = out_p.rearrange("(c j p) o -> c p j o", j=CPB, p=P)

    with tile.TileContext(nc) as tc, ExitStack() as ctx:
        const = ctx.enter_context(tc.tile_pool(name="const", bufs=1))
        wpool = ctx.enter_context(tc.tile_pool(name="wpool", bufs=1))
        kvpool = ctx.enter_context(tc.tile_pool(name="kvpool", bufs=1))
        xt_pool = ctx.enter_context(tc.tile_pool(name="xtp", bufs=2))
        xq_pool = ctx.enter_context(tc.tile_pool(name="xqp", bufs=16))
        trig_pool = ctx.enter_context(tc.tile_pool(name="trigp", bufs=2))
        trigs_pool = ctx.enter_context(tc.tile_pool(name="trigsp", bufs=2))
        sc_pool = ctx.enter_context(tc.tile_pool(name="scp", bufs=2))
        bc_pool = ctx.enter_context(tc.tile_pool(name="bcp", bufs=2))
        rope_t = ctx.enter_context(tc.tile_pool(name="ropet", bufs=2))
        qk_pool = ctx.enter_context(tc.tile_pool(name="qkev", bufs=8))
        q_pool = ctx.enter_context(tc.tile_pool(name="qp", bufs=4))
        ex_pool = ctx.enter_context(tc.tile_pool(name="exp", bufs=4))
        ctx_pool = ctx.enter_context(tc.tile_pool(name="ctxp", bufs=2))
        o_pool = ctx.enter_context(tc.tile_pool(name="op", bufs=3))
        ps = ctx.enter_context(tc.tile_pool(name="ps", bufs=8, space="PSUM"))

        # ---- small constants (cheap DMAs first) ----
        ones_sb = const.tile([P, 1], BF16)
        nc.sync.dma_start(out=ones_sb, in_=onesT)
        mask_sb = const.tile([P, P], BF16)
        nc.sync.dma_start(out=mask_sb, in_=maskT)
        identB = const.tile([P, P], BF16)
        nc.sync.dma_start(out=identB, in_=identT)
        ident = const.tile([P, P], F32)
        make_identity(nc, ident)
        eps_sb = const.tile([P, 1], F32)
        nc.vector.memset(eps_sb, RMS_EPS)

        wq_sb = wpool.tile([P, KT, OC], BF16)
        wk_sb = wpool.tile([P, KT, OC], BF16)
        wv_sb = wpool.tile([P, KT, OC], BF16)
        wo_sb = wpool.tile([P, HPC, H], BF16)

        # per-chunk K/V caches, resident for the whole kernel
        k_chunks = [kvpool.tile([P, HPC, CH], BF16, name=f"kc{i}") for i in range(NCH)]
        v_chunks = [kvpool.tile([P, CPB, OC], BF16, name=f"vc{i}") for i in range(NCH)]

        cs = [dict() for _ in range(NCH)]   # per-chunk pipeline state

        def emit_loads(i):
            st = cs[i]
            st["xt"] = xt_pool.tile([P, KT, CH], BF16, tag="xta", name=f"xt{i}")
            nc.sync.dma_start(out=st["xt"], in_=xTr[:, :, i * CH:(i + 1) * CH])
            st["trig"] = trig_pool.tile([P, 2, CH], BF16, tag="trig", name=f"tg{i}")
            nc.sync.dma_start(out=st["trig"], in_=trig_r[:, :, i * CH:(i + 1) * CH])

        def emit_passA(i):
            st = cs[i]
            xt_all = st["xt"]
            qp = [ps.tile([P, CH], F32, tag="ps", name=f"qp{i}_{h}") for h in range(HPC)]
            kp = [ps.tile([P, CH], F32, tag="ps", name=f"kp{i}_{h}") for h in range(HPC)]
            vp = [ps.tile([P, 2, OC], F32, tag="ps", name=f"vp{i}_{g}") for g in range(2)]
            stats = ps.tile([P, CPB], F32, tag="ps", name=f"ss{i}")
            xqs = []
            for dt in range(KT):
                xq = xq_pool.tile([P, CH], BF16, tag="xq", name=f"xq{i}_{dt}")
                nc.vector.tensor_mul(xq, xt_all[:, dt, :], xt_all[:, dt, :])
                xqs.append(xq)
            for dt in range(KT):
                st_, sp_ = (dt == 0), (dt == KT - 1)
                for h in range(HPC):
                    nc.tensor.matmul(qp[h], wq_sb[:, dt, h * P:(h + 1) * P],
                                     xt_all[:, dt, :], start=st_, stop=sp_)
            for dt in range(KT):
                st_, sp_ = (dt == 0), (dt == KT - 1)
                for h in range(HPC):
                    nc.tensor.matmul(kp[h], wk_sb[:, dt, h * P:(h + 1) * P],
                                     xt_all[:, dt, :], start=st_, stop=sp_)
            for dt in range(KT):
                st_, sp_ = (dt == 0), (dt == KT - 1)
                for j in range(CPB):
                    nc.tensor.matmul(vp[j // 2][:, j % 2, :],
                                     xt_all[:, dt, j * P:(j + 1) * P],
                                     wv_sb[:, dt, :], start=st_, stop=sp_,
                                     skip_group_check=True)
            for dt in range(KT):
                for j in range(CPB):
                    nc.tensor.matmul(stats[:, j:j + 1], xqs[dt][:, j * P:(j + 1) * P],
                                     ones_sb, start=(dt == 0), stop=(dt == KT - 1),
                                     skip_group_check=True)
            qe = []
            for h in range(HPC):
                for src_p in (qp[h], kp[h]):
                    ev = qk_pool.tile([P, CH], F32, tag="qke", name=f"qe{i}_{len(qe)}")
                    nc.vector.tensor_copy(ev, src_p)
                    qe.append(ev)
            st.update(qe=qe, vp=vp, stats=stats)

        def emit_rms_vevict(i):
            st = cs[i]
            lnv = sc_pool.tile([P, CPB], F32, tag="ln", name=f"ln{i}")
            nc.scalar.activation(lnv, st["stats"], LN, bias=eps_sb, scale=1.0 / H)
            s_col = sc_pool.tile([P, CPB], F32, tag="sc", name=f"scol{i}")
            nc.scalar.activation(s_col, lnv, EXP, scale=-0.5)
            for j in range(CPB):
                nc.scalar.activation(v_chunks[i][:, j, :], st["vp"][j // 2][:, j % 2, :],
                                     COPY, scale=s_col[:, j:j + 1])
            st["s_col"] = s_col

        def emit_s_transpose(i):
            st = cs[i]
            st["s_rowT_p"] = ps.tile([CPB, P], F32, tag="ps", name=f"srt{i}")
            nc.tensor.transpose(st["s_rowT_p"], st["s_col"], ident)

        def emit_trig_rope(i):
            st = cs[i]
            s_rowT_p = st["s_rowT_p"]
            s_row = sc_pool.tile([1, CH], F32, tag="sr", name=f"srow{i}")
            for j in range(CPB):
                nc.vector.tensor_copy(s_row[0:1, j * P:(j + 1) * P], s_rowT_p[j:j + 1, :])
            s_bc = bc_pool.tile([P, CH], F32, tag="sbc", name=f"sbc{i}")
            nc.gpsimd.partition_broadcast(s_bc, s_row)
            cosS = trigs_pool.tile([P, CH], F32, tag="cosS", name=f"cs{i}")
            nc.vector.tensor_mul(cosS, st["trig"][:, 0, :], s_bc)
            sinS = trigs_pool.tile([P, CH], F32, tag="sinS", name=f"sn{i}")
            nc.vector.tensor_mul(sinS, st["trig"][:, 1, :], s_bc)
            q_sb = []
            for h in range(HPC):
                for (psum_t, dst) in ((st["qe"][2 * h], None), (st["qe"][2 * h + 1], k_chunks[i][:, h, :])):
                    t1 = rope_t.tile([P, CH], F32, tag="t1", name=f"t1_{i}")
                    nc.vector.tensor_mul(t1, psum_t, cosS)
                    t2 = rope_t.tile([P, CH], F32, tag="t2", name=f"t2_{i}")
                    nc.vector.tensor_mul(t2[0:HD, :], psum_t[HD:P, :], sinS[0:HD, :])
                    nc.vector.tensor_mul(t2[HD:P, :], psum_t[0:HD, :], sinS[HD:P, :])
                    if dst is None:
                        dst = q_pool.tile([P, CH], BF16, tag="q", name=f"q{i}_{h}")
                        q_sb.append(dst)
                    nc.vector.tensor_add(dst, t1, t2)
            st["q_sb"] = q_sb

        def emit_attn(i):
            st = cs[i]
            b, li = i // CPB, i % CPB
            nkt = CPB * (li + 1)
            ctx_sb = ctx_pool.tile([P, HPC, CH], BF16, tag="ctx", name=f"ctx{i}")
            den = ps.tile([P, HPC * CPB], F32, tag="ps", name=f"dn{i}")
            ctxs, rec_cols = [], []
            for h in range(HPC):
                ctxp = ps.tile([P, CH], F32, tag="ps", name=f"cx{i}_{h}")
                ctxs.append(ctxp)
                pend = []

                def flush_one(h=h, ctxp=ctxp):
                    kt0, qa0, ex0, ck0, j0 = pend.pop(0)
                    nc.tensor.matmul(ctxp[:, qa0:], v_chunks[ck0][:, j0, h * P:(h + 1) * P],
                                     ex0[:, qa0:], start=(kt0 == 0), stop=(kt0 == nkt - 1),
                                     skip_group_check=True)
                    kk0 = kt0 - CPB * li
                    for j2 in range(CPB):
                        if kk0 <= j2:
                            nc.tensor.matmul(den[:, h * CPB + j2:h * CPB + j2 + 1],
                                             ex0[:, j2 * P:(j2 + 1) * P], ones_sb,
                                             start=(kt0 == 0), stop=(kt0 == CPB * li + j2),
                                             skip_group_check=True)

                for kt in range(nkt):
                    ck = b * CPB + kt // CPB
                    j = kt % CPB
                    kk = kt - CPB * li
                    qa = kk * P if kk > 0 else 0
                    sp_ = ps.tile([P, CH], F32, tag="ps", name=f"s{i}_{h}_{kt}")
                    diag = kk >= 0
                    nc.tensor.matmul(sp_[:, qa:], k_chunks[ck][:, h, j * P:(j + 1) * P],
                                     st["q_sb"][h][:, qa:], start=True, stop=not diag,
                                     skip_group_check=True)
                    if diag:
                        nc.tensor.matmul(sp_[:, kk * P:(kk + 1) * P], mask_sb, identB,
                                         start=False, stop=True, skip_group_check=True)
                    ex = ex_pool.tile([P, CH], BF16, tag="ex", name=f"ex{i}_{h}_{kt}")
                    nc.scalar.activation(ex[:, qa:], sp_[:, qa:], EXP, scale=SCALE)
                    pend.append((kt, qa, ex, ck, j))
                    if len(pend) > 2:
                        flush_one()
                while pend:
                    flush_one()
                rec_col = sc_pool.tile([P, CPB], F32, tag="rc", name=f"rc{i}_{h}")
                nc.vector.reciprocal(rec_col, den[:, h * CPB:(h + 1) * CPB])
                rec_cols.append(rec_col)

            for h in range(HPC):
                recT_p = ps.tile([CPB, P], F32, tag="ps", name=f"rt{i}_{h}")
                nc.tensor.transpose(recT_p, rec_cols[h], ident)
                rec_row = sc_pool.tile([1, CH], F32, tag="rt", name=f"rct{i}_{h}")
                for j in range(CPB):
                    nc.vector.tensor_copy(rec_row[0:1, j * P:(j + 1) * P], recT_p[j:j + 1, :])
                rbc = bc_pool.tile([P, CH], F32, tag="rbc", name=f"rbc{i}_{h}")
                nc.gpsimd.partition_broadcast(rbc, rec_row)
                nc.vector.tensor_mul(ctx_sb[:, h, :], ctxs[h], rbc)
            st["ctx_sb"] = ctx_sb

        def emit_oproj(i, inject=None):
            csb = cs[i]["ctx_sb"]
            for j in range(CPB):
                o_sb = o_pool.tile([P, H], BF16, tag="osb", name=f"ob{i}_{j}")
                for oc in range(H // CH):
                    op_ = ps.tile([P, CH], F32, tag="ps", name=f"o{i}_{j}_{oc}")
                    for h in range(HPC):
                        nc.tensor.matmul(op_, csb[:, h, j * P:(j + 1) * P],
                                         wo_sb[:, h, oc * CH:(oc + 1) * CH],
                                         start=(h == 0), stop=(h == HPC - 1))
                    nc.scalar.copy(o_sb[:, oc * CH:(oc + 1) * CH], op_)
                nc.scalar.dma_start(out=out_pr[i][:, j, :], in_=o_sb)
                if j == 0 and inject is not None:
                    inject()

        # ---- prologue: overlap weight loads with the first chunk's input ----
        nc.sync.dma_start(out=wq_sb, in_=wqT.rearrange("(t p) o -> p t o", p=P))
        emit_loads(0)
        nc.sync.dma_start(out=wk_sb, in_=wkT.rearrange("(t p) o -> p t o", p=P))
        emit_loads(1)
        nc.sync.dma_start(out=wv_sb, in_=wvT.rearrange("(t p) o -> p t o", p=P))
        nc.sync.dma_start(out=wo_sb, in_=woT.rearrange("(t p) o -> p t o", p=P))

        emit_passA(0)
        emit_rms_vevict(0)
        emit_s_transpose(0)
        emit_trig_rope(0)
        for i in range(NCH):
            if i + 1 < NCH:
                emit_passA(i + 1)
                emit_rms_vevict(i + 1)
            if i >= 1:
                emit_oproj(i - 1,
                           inject=(lambda k=i + 1: emit_s_transpose(k)) if i + 1 < NCH else None)
            elif i + 1 < NCH:
                emit_s_transpose(i + 1)
            if i + 1 < NCH:
                emit_trig_rope(i + 1)
                if i + 2 < NCH:
                    emit_loads(i + 2)
            emit_attn(i)
        emit_oproj(NCH - 1)

    nc.compile()
    return nc


def prep_inputs(x, norm_w, wq, wk, wv, wo, position_ids):
    """Host-side sharding/layout prep. Returns per-core input maps."""
    import ml_dtypes
    bf16 = ml_dtypes.bfloat16
    x = np.asarray(x, dtype=np.float32)
    norm_w = np.asarray(norm_w, dtype=np.float32)
    wq = np.asarray(wq, dtype=np.float32)
    wk = np.asarray(wk, dtype=np.float32)
    wv = np.asarray(wv, dtype=np.float32)
    wo = np.asarray(wo, dtype=np.float32)
    pos = np.asarray(position_ids)

    xT = np.ascontiguousarray(x.reshape(NT, H).T).astype(bf16)

    # RoPE tables from position_ids, sign-folded sin
    inv_freq = 1.0 / (ROPE_BASE ** (np.arange(0, D, 2, dtype=np.float32) / D))
    t = pos.reshape(NT).astype(np.float32)
    freqs = np.einsum("n,f->nf", t, inv_freq)
    emb = np.concatenate([freqs, freqs], axis=1)          # [NT, D]
    cos = np.cos(emb).astype(np.float32)
    sin = np.sin(emb).astype(np.float32)
    sinF = sin.copy()
    sinF[:, :HD] *= -1.0
    trigT = np.stack([np.ascontiguousarray(cos.T),
                      np.ascontiguousarray(sinF.T)]).astype(bf16)   # [2, D, NT]

    # transposed triangle mask, added to scores via maskT.T @ I on the PE:
    # MT[qq, kp] = -1e4 where qq < kp (strictly-upper triangle)
    rr = np.arange(P)[:, None]
    cc = np.arange(P)[None, :]
    maskT = np.where(rr < cc, MASK_VAL, 0.0).astype(bf16)

    onesT = np.ones((P, 1), dtype=bf16)
    identT = np.eye(P, dtype=np.float32).astype(bf16)

    wq_f = wq * norm_w[None, :]
    wk_f = wk * norm_w[None, :]
    wv_f = wv * norm_w[None, :]

    in_maps = []
    for c in range(NCORES):
        sl = slice(c * OC, (c + 1) * OC)
        in_maps.append({
            "xT": xT,
            "wqT": np.ascontiguousarray(wq_f[sl].T).astype(bf16),
            "wkT": np.ascontiguousarray(wk_f[sl].T).astype(bf16),
            "wvT": np.ascontiguousarray(wv_f[sl].T).astype(bf16),
            "woT": np.ascontiguousarray(wo[:, sl].T).astype(bf16),
            "trigT": trigT,
            "maskT": maskT,
            "onesT": onesT,
            "identT": identT,
        })
    return in_maps


_NC_CACHE = None


def _get_module():
    global _NC_CACHE
    if _NC_CACHE is None:
        _NC_CACHE = build_module()
    return _NC_CACHE


def kernel(x, norm_w, wq, wk, wv, wo, position_ids):
    nc = _get_module()
    in_maps = prep_inputs(x, norm_w, wq, wk, wv, wo, position_ids)
    res = bass_utils.run_bass_kernel_spmd(nc, in_maps, core_ids=list(range(NCORES)))
    acc = np.zeros((NT, H), dtype=np.float32)
    for c in range(NCORES):
        acc += res.results[c]["out_p"].astype(np.float32)
    return acc.reshape(B, S, H)


# revision 12
# speedup vs baseline: 1.0030x; 1.0030x over previous
"""DeepSpeed-style self-attention block (RMSNorm + QKV + RoPE + causal attention
+ output projection) on 8 Trainium2 NeuronCores.

Sharding: tensor-parallel over heads (16 heads -> 2 per core). Each core computes
its 2 heads' attention over the full sequence and a partial output projection over
its 256-dim slice of the context; the 8 partial outputs are summed on the host
(the TP all-reduce equivalent, done at gather time).

All PE matmuls run in bfloat16 (1 cyc/row at any ap size). Per-token RMS stats and
softmax denominators are computed with ap=1 matmuls (ones moving tensor, data as
stationary), which cost ~nothing on the PE. V is projected directly into natural
[token, dv] layout (x stationary, weights moving) so no PE transposes or extra
copies are needed; the RMS scale s is applied at V eviction via the activation
engine's per-partition scale operand. rsqrt is computed as exp(-0.5*ln(x)).
Scores/pv/exp are trimmed to the causal region at 128-column granularity, and the
causal mask shrinks to a single 128x128 triangle tile applied only on the exact
diagonal blocks.

Schedule (software pipelined, one iteration per chunk i):
  PE:  oproj(i-1) | passA(i+1) [q-phase, k-phase, v-phase, stats] | attn(i)
The RMS->RoPE chain of chunk i+1 executes on Act/Pool/DVE underneath oproj(i)/
passA(i+2), so the PE never waits for q/k eviction. DMA is batched (2 loads +
4 quarter-stores per 512-token chunk); stores issue from the activation engine's
queue right after their evictions.
"""
import sys
sys.path.insert(0, '/opt/trn_rl_repo')

import math
import numpy as np
from contextlib import ExitStack

import concourse.bass as bass
from concourse import bacc
import concourse.mybir as mybir
import concourse.tile as tile
from concourse import bass_utils
from concourse.masks import make_identity

# ---- problem constants (hardcoded per contest contract) ----
B, S, H, HEADS, D = 2, 2048, 2048, 16, 128
NT = B * S                    # 4096 tokens
NCORES = 8
HPC = HEADS // NCORES         # 2 heads per core
OC = HPC * D                  # 256 output dims per core
P = 128
CH = 512                      # token chunk
NCH = NT // CH                # 8 chunks
KT = H // P                   # 16 d-tiles
CPB = S // CH                 # 4 chunks per batch
HD = D // 2
SCALE = 1.0 / math.sqrt(D)
RMS_EPS = 1e-6
ROPE_BASE = 10000.0
MASK_VAL = -10000.0

F32 = mybir.dt.float32
BF16 = mybir.dt.bfloat16
EXP = mybir.ActivationFunctionType.Exp
LN = mybir.ActivationFunctionType.Ln
COPY = mybir.ActivationFunctionType.Copy


def build_module():
    nc = bacc.Bacc("TRN2", target_bir_lowering=False, debug=False, num_devices=NCORES)

    xT = nc.dram_tensor("xT", [H, NT], BF16, kind="ExternalInput").ap()
    wqT = nc.dram_tensor("wqT", [H, OC], BF16, kind="ExternalInput").ap()
    wkT = nc.dram_tensor("wkT", [H, OC], BF16, kind="ExternalInput").ap()
    wvT = nc.dram_tensor("wvT", [H, OC], BF16, kind="ExternalInput").ap()
    woT = nc.dram_tensor("woT", [OC, H], BF16, kind="ExternalInput").ap()
    trigT = nc.dram_tensor("trigT", [2, D, NT], BF16, kind="ExternalInput").ap()
    maskT = nc.dram_tensor("maskT", [P, P], BF16, kind="ExternalInput").ap()
    onesT = nc.dram_tensor("onesT", [P, 1], BF16, kind="ExternalInput").ap()
    identT = nc.dram_tensor("identT", [P, P], BF16, kind="ExternalInput").ap()
    out_p = nc.dram_tensor("out_p", [NT, H], BF16, kind="ExternalOutput").ap()

    xTr = xT.rearrange("(t p) n -> p t n", p=P)
    trig_r = trigT.rearrange("s d n -> d s n")
    out_pr = out_p.rearrange("(c j p) o -> c p j o", j=CPB, p=P)

    with tile.TileContext(nc) as tc, ExitStack() as ctx:
        const = ctx.enter_context(tc.tile_pool(name="const", bufs=1))
        wpool = ctx.enter_context(tc.tile_pool(name="wpool", bufs=1))
        kvpool = ctx.enter_context(tc.tile_pool(name="kvpool", bufs=1))
        xt_pool = ctx.enter_context(tc.tile_pool(name="xtp", bufs=2))
        xq_pool = ctx.enter_context(tc.tile_pool(name="xqp", bufs=16))
        trig_pool = ctx.enter_context(tc.tile_pool(name="trigp", bufs=2))
        trigs_pool = ctx.enter_context(tc.tile_pool(name="trigsp", bufs=2))
        sc_pool = ctx.enter_context(tc.tile_pool(name="scp", bufs=2))
        bc_pool = ctx.enter_context(tc.tile_pool(name="bcp", bufs=2))
        rope_t = ctx.enter_context(tc.tile_pool(name="ropet", bufs=2))
        qk_pool = ctx.enter_context(tc.tile_pool(name="qkev", bufs=8))
        q_pool = ctx.enter_context(tc.tile_pool(name="qp", bufs=4))
        ex_pool = ctx.enter_context(tc.tile_pool(name="exp", bufs=4))
        ctx_pool = ctx.enter_context(tc.tile_pool(name="ctxp", bufs=2))
        o_pool = ctx.enter_context(tc.tile_pool(name="op", bufs=3))
        ps = ctx.enter_context(tc.tile_pool(name="ps", bufs=8, space="PSUM"))

        # ---- small constants (cheap DMAs first) ----
        ones_sb = const.tile([P, 1], BF16)
        nc.sync.dma_start(out=ones_sb, in_=onesT)
        mask_sb = const.tile([P, P], BF16)
        nc.sync.dma_start(out=mask_sb, in_=maskT)
        identB = const.tile([P, P], BF16)
        nc.sync.dma_start(out=identB, in_=identT)
        ident = const.tile([P, P], F32)
        make_identity(nc, ident)
        eps_sb = const.tile([P, 1], F32)
        nc.vector.memset(eps_sb, RMS_EPS)

        wq_sb = wpool.tile([P, KT, OC], BF16)
        wk_sb = wpool.tile([P, KT, OC], BF16)
        wv_sb = wpool.tile([P, KT, OC], BF16)
        wo_sb = wpool.tile([P, HPC, H], BF16)

        # per-chunk K/V caches, resident for the whole kernel
        k_chunks = [kvpool.tile([P, HPC, CH], BF16, name=f"kc{i}") for i in range(NCH)]
        v_chunks = [kvpool.tile([P, CPB, OC], BF16, name=f"vc{i}") for i in range(NCH)]

        cs = [dict() for _ in range(NCH)]   # per-chunk pipeline state

        def emit_loads(i):
            st = cs[i]
            st["xt"] = xt_pool.tile([P, KT, CH], BF16, tag="xta", name=f"xt{i}")
            nc.sync.dma_start(out=st["xt"], in_=xTr[:, :, i * CH:(i + 1) * CH])
            st["trig"] = trig_pool.tile([P, 2, CH], BF16, tag="trig", name=f"tg{i}")
            nc.sync.dma_start(out=st["trig"], in_=trig_r[:, :, i * CH:(i + 1) * CH])

        def emit_passA(i):
            st = cs[i]
            xt_all = st["xt"]
            qp = [ps.tile([P, CH], F32, tag="ps", name=f"qp{i}_{h}") for h in range(HPC)]
            kp = [ps.tile([P, CH], F32, tag="ps", name=f"kp{i}_{h}") for h in range(HPC)]
            vp = [ps.tile([P, 2, OC], F32, tag="ps", name=f"vp{i}_{g}") for g in range(2)]
            stats = ps.tile([P, CPB], F32, tag="ps", name=f"ss{i}")
            xqs = []
            for dt in range(KT):
                xq = xq_pool.tile([P, CH], BF16, tag="xq", name=f"xq{i}_{dt}")
                nc.vector.tensor_mul(xq, xt_all[:, dt, :], xt_all[:, dt, :])
                xqs.append(xq)
            for dt in range(KT):
                st_, sp_ = (dt == 0), (dt == KT - 1)
                for h in range(HPC):
                    nc.tensor.matmul(qp[h], wq_sb[:, dt, h * P:(h + 1) * P],
                                     xt_all[:, dt, :], start=st_, stop=sp_)
            for dt in range(KT):
                st_, sp_ = (dt == 0), (dt == KT - 1)
                for h in range(HPC):
                    nc.tensor.matmul(kp[h], wk_sb[:, dt, h * P:(h + 1) * P],
                                     xt_all[:, dt, :], start=st_, stop=sp_)
            for dt in range(KT):
                st_, sp_ = (dt == 0), (dt == KT - 1)
                for j in range(CPB):
                    nc.tensor.matmul(vp[j // 2][:, j % 2, :],
                                     xt_all[:, dt, j * P:(j + 1) * P],
                                     wv_sb[:, dt, :], start=st_, stop=sp_,
                                     skip_group_check=True)
            for dt in range(KT):
                for j in range(CPB):
                    nc.tensor.matmul(stats[:, j:j + 1], xqs[dt][:, j * P:(j + 1) * P],
                                     ones_sb, start=(dt == 0), stop=(dt == KT - 1),
                                     skip_group_check=True)
            st.update(qe=[qp[0], kp[0], qp[1], kp[1]], vp=vp, stats=stats)

        def emit_rms_vevict(i):
            st = cs[i]
            lnv = sc_pool.tile([P, CPB], F32, tag="ln", name=f"ln{i}")
            nc.scalar.activation(lnv, st["stats"], LN, bias=eps_sb, scale=1.0 / H)
            s_col = sc_pool.tile([P, CPB], F32, tag="sc", name=f"scol{i}")
            nc.scalar.activation(s_col, lnv, EXP, scale=-0.5)
            for j in range(CPB):
                nc.scalar.activation(v_chunks[i][:, j, :], st["vp"][j // 2][:, j % 2, :],
                                     COPY, scale=s_col[:, j:j + 1])
            st["s_col"] = s_col

        def emit_s_transpose(i):
            st = cs[i]
            st["s_rowT_p"] = ps.tile([CPB, P], F32, tag="ps", name=f"srt{i}")
            nc.tensor.transpose(st["s_rowT_p"], st["s_col"], ident)

        def emit_trig_rope(i):
            st = cs[i]
            s_rowT_p = st["s_rowT_p"]
            s_row = sc_pool.tile([1, CH], F32, tag="sr", name=f"srow{i}")
            for j in range(CPB):
                nc.vector.tensor_copy(s_row[0:1, j * P:(j + 1) * P], s_rowT_p[j:j + 1, :])
            s_bc = bc_pool.tile([P, CH], F32, tag="sbc", name=f"sbc{i}")
            nc.gpsimd.partition_broadcast(s_bc, s_row)
            cosS = trigs_pool.tile([P, CH], F32, tag="cosS", name=f"cs{i}")
            nc.vector.tensor_mul(cosS, st["trig"][:, 0, :], s_bc)
            sinS = trigs_pool.tile([P, CH], F32, tag="sinS", name=f"sn{i}")
            nc.vector.tensor_mul(sinS, st["trig"][:, 1, :], s_bc)
            q_sb = []
            for h in range(HPC):
                for (psum_t, dst) in ((st["qe"][2 * h], None), (st["qe"][2 * h + 1], k_chunks[i][:, h, :])):
                    t1 = rope_t.tile([P, CH], F32, tag="t1", name=f"t1_{i}")
                    nc.vector.tensor_mul(t1, psum_t, cosS)
                    t2 = rope_t.tile([P, CH], F32, tag="t2", name=f"t2_{i}")
                    nc.vector.tensor_mul(t2[0:HD, :], psum_t[HD:P, :], sinS[0:HD, :])
                    nc.vector.tensor_mul(t2[HD:P, :], psum_t[0:HD, :], sinS[HD:P, :])
                    if dst is None:
                        dst = q_pool.tile([P, CH], BF16, tag="q", name=f"q{i}_{h}")
                        q_sb.append(dst)
                    nc.vector.tensor_add(dst, t1, t2)
            st["q_sb"] = q_sb

        def emit_attn(i):
            st = cs[i]
            b, li = i // CPB, i % CPB
            nkt = CPB * (li + 1)
            ctx_sb = ctx_pool.tile([P, HPC, CH], BF16, tag="ctx", name=f"ctx{i}")
            den = ps.tile([P, HPC * CPB], F32, tag="ps", name=f"dn{i}")
            ctxs, rec_cols = [], []
            for h in range(HPC):
                ctxp = ps.tile([P, CH], F32, tag="ps", name=f"cx{i}_{h}")
                ctxs.append(ctxp)
                pend = []

                def flush_one(h=h, ctxp=ctxp):
                    kt0, qa0, ex0, ck0, j0 = pend.pop(0)
                    nc.tensor.matmul(ctxp[:, qa0:], v_chunks[ck0][:, j0, h * P:(h + 1) * P],
                                     ex0[:, qa0:], start=(kt0 == 0), stop=(kt0 == nkt - 1),
                                     skip_group_check=True)
                    kk0 = kt0 - CPB * li
                    for j2 in range(CPB):
                        if kk0 <= j2:
                            nc.tensor.matmul(den[:, h * CPB + j2:h * CPB + j2 + 1],
                                             ex0[:, j2 * P:(j2 + 1) * P], ones_sb,
                                             start=(kt0 == 0), stop=(kt0 == CPB * li + j2),
                                             skip_group_check=True)

                for kt in range(nkt):
                    ck = b * CPB + kt // CPB
                    j = kt % CPB
                    kk = kt - CPB * li
                    qa = kk * P if kk > 0 else 0
                    sp_ = ps.tile([P, CH], F32, tag="ps", name=f"s{i}_{h}_{kt}")
                    diag = kk >= 0
                    nc.tensor.matmul(sp_[:, qa:], k_chunks[ck][:, h, j * P:(j + 1) * P],
                                     st["q_sb"][h][:, qa:], start=True, stop=not diag,
                                     skip_group_check=True)
                    if diag:
                        nc.tensor.matmul(sp_[:, kk * P:(kk + 1) * P], mask_sb, identB,
                                         start=False, stop=True, skip_group_check=True)
                    ex = ex_pool.tile([P, CH], BF16, tag="ex", name=f"ex{i}_{h}_{kt}")
                    nc.scalar.activation(ex[:, qa:], sp_[:, qa:], EXP, scale=SCALE)
                    pend.append((kt, qa, ex, ck, j))
                    if len(pend) > 2:
                        flush_one()
                while pend:
                    flush_one()
                rec_col = sc_pool.tile([P, CPB], F32, tag="rc", name=f"rc{i}_{h}")
                nc.vector.reciprocal(rec_col, den[:, h * CPB:(h + 1) * CPB])
                rec_cols.append(rec_col)

            for h in range(HPC):
                recT_p = ps.tile([CPB, P], F32, tag="ps", name=f"rt{i}_{h}")
                nc.tensor.transpose(recT_p, rec_cols[h], ident)
                rec_row = sc_pool.tile([1, CH], F32, tag="rt", name=f"rct{i}_{h}")
                for j in range(CPB):
                    nc.vector.tensor_copy(rec_row[0:1, j * P:(j + 1) * P], recT_p[j:j + 1, :])
                rbc = bc_pool.tile([P, CH], F32, tag="rbc", name=f"rbc{i}_{h}")
                nc.gpsimd.partition_broadcast(rbc, rec_row)
                nc.vector.tensor_mul(ctx_sb[:, h, :], ctxs[h], rbc)
            st["ctx_sb"] = ctx_sb

        def emit_oproj(i, inject=None):
            csb = cs[i]["ctx_sb"]
            for j in range(CPB):
                o_sb = o_pool.tile([P, H], BF16, tag="osb", name=f"ob{i}_{j}")
                for oc in range(H // CH):
                    op_ = ps.tile([P, CH], F32, tag="ps", name=f"o{i}_{j}_{oc}")
                    for h in range(HPC):
                        nc.tensor.matmul(op_, csb[:, h, j * P:(j + 1) * P],
                                         wo_sb[:, h, oc * CH:(oc + 1) * CH],
                                         start=(h == 0), stop=(h == HPC - 1))
                    nc.scalar.copy(o_sb[:, oc * CH:(oc + 1) * CH], op_)
                nc.scalar.dma_start(out=out_pr[i][:, j, :], in_=o_sb)
                if j == 0 and inject is not None:
                    inject()

        # ---- prologue: overlap weight loads with the first chunk's input ----
        nc.sync.dma_start(out=wq_sb, in_=wqT.rearrange("(t p) o -> p t o", p=P))
        emit_loads(0)
        nc.sync.dma_start(out=wk_sb, in_=wkT.rearrange("(t p) o -> p t o", p=P))
        emit_loads(1)
        nc.sync.dma_start(out=wv_sb, in_=wvT.rearrange("(t p) o -> p t o", p=P))
        nc.sync.dma_start(out=wo_sb, in_=woT.rearrange("(t p) o -> p t o", p=P))

        emit_passA(0)
        emit_rms_vevict(0)
        emit_s_transpose(0)
        emit_trig_rope(0)
        for i in range(NCH):
            if i + 1 < NCH:
                emit_passA(i + 1)
                emit_rms_vevict(i + 1)
            if i >= 1:
                emit_oproj(i - 1,
                           inject=(lambda k=i + 1: emit_s_transpose(k)) if i + 1 < NCH else None)
            elif i + 1 < NCH:
                emit_s_transpose(i + 1)
            if i + 1 < NCH:
                emit_trig_rope(i + 1)
                if i + 2 < NCH:
                    emit_loads(i + 2)
            emit_attn(i)
        emit_oproj(NCH - 1)

    nc.compile()
    return nc


def prep_inputs(x, norm_w, wq, wk, wv, wo, position_ids):
    """Host-side sharding/layout prep. Returns per-core input maps."""
    import ml_dtypes
    bf16 = ml_dtypes.bfloat16
    x = np.asarray(x, dtype=np.float32)
    norm_w = np.asarray(norm_w, dtype=np.float32)
    wq = np.asarray(wq, dtype=np.float32)
    wk = np.asarray(wk, dtype=np.float32)
    wv = np.asarray(wv, dtype=np.float32)
    wo = np.asarray(wo, dtype=np.float32)
    pos = np.asarray(position_ids)

    xT = np.ascontiguousarray(x.reshape(NT, H).T).astype(bf16)

    # RoPE tables from position_ids, sign-folded sin
    inv_freq = 1.0 / (ROPE_BASE ** (np.arange(0, D, 2, dtype=np.float32) / D))
    t = pos.reshape(NT).astype(np.float32)
    freqs = np.einsum("n,f->nf", t, inv_freq)
    emb = np.concatenate([freqs, freqs], axis=1)          # [NT, D]
    cos = np.cos(emb).astype(np.float32)
    sin = np.sin(emb).astype(np.float32)
    sinF = sin.copy()
    sinF[:, :HD] *= -1.0
    trigT = np.stack([np.ascontiguousarray(cos.T),
                      np.ascontiguousarray(sinF.T)]).astype(bf16)   # [2, D, NT]

    # transposed triangle mask, added to scores via maskT.T @ I on the PE:
    # MT[qq, kp] = -1e4 where qq < kp (strictly-upper triangle)
    rr = np.arange(P)[:, None]
    cc = np.arange(P)[None, :]
    maskT = np.where(rr < cc, MASK_VAL, 0.0).astype(bf16)

    onesT = np.ones((P, 1), dtype=bf16)
    identT = np.eye(P, dtype=np.float32).astype(bf16)

    wq_f = wq * norm_w[None, :]
    wk_f = wk * norm_w[None, :]
    wv_f = wv * norm_w[None, :]

    in_maps = []
    for c in range(NCORES):
        sl = slice(c * OC, (c + 1) * OC)
        in_maps.append({
            "xT": xT,
            "wqT": np.ascontiguousarray(wq_f[sl].T).astype(bf16),
            "wkT": np.ascontiguousarray(wk_f[sl].T).astype(bf16),
            "wvT": np.ascontiguousarray(wv_f[sl].T).astype(bf16),
            "woT": np.ascontiguousarray(wo[:, sl].T).astype(bf16),
            "trigT": trigT,
            "maskT": maskT,
            "onesT": onesT,
            "identT": identT,
        })
    return in_maps


_NC_CACHE = None


def _get_module():
    global _NC_CACHE
    if _NC_CACHE is None:
        _NC_CACHE = build_module()
    return _NC_CACHE


def kernel(x, norm_w, wq, wk, wv, wo, position_ids):
    nc = _get_module()
    in_maps = prep_inputs(x, norm_w, wq, wk, wv, wo, position_ids)
    res = bass_utils.run_bass_kernel_spmd(nc, in_maps, core_ids=list(range(NCORES)))
    acc = np.zeros((NT, H), dtype=np.float32)
    for c in range(NCORES):
        acc += res.results[c]["out_p"].astype(np.float32)
    return acc.reshape(B, S, H)


# revision 13
# speedup vs baseline: 1.0195x; 1.0165x over previous
"""DeepSpeed-style self-attention block (RMSNorm + QKV + RoPE + causal attention
+ output projection) on 8 Trainium2 NeuronCores.

Sharding: tensor-parallel over heads (16 heads -> 2 per core). Each core computes
its 2 heads' attention over the full sequence and a partial output projection over
its 256-dim slice of the context; the 8 partial outputs are summed on the host
(the TP all-reduce equivalent, done at gather time).

All PE matmuls run in bfloat16 (1 cyc/row at any ap size). Per-token RMS stats and
softmax denominators are computed with ap=1 matmuls (ones moving tensor, data as
stationary), which cost ~nothing on the PE. V is projected directly into natural
[token, dv] layout (x stationary, weights moving) so no PE transposes or extra
copies are needed; the RMS scale s is applied at V eviction via the activation
engine's per-partition scale operand. rsqrt is computed as exp(-0.5*ln(x)).
Scores/pv/exp are trimmed to the causal region at 128-column granularity, and the
causal mask shrinks to a single 128x128 triangle tile applied only on the exact
diagonal blocks.

Schedule (software pipelined, one iteration per chunk i):
  PE:  oproj(i-1) | passA(i+1) [q-phase, k-phase, v-phase, stats] | attn(i)
The RMS->RoPE chain of chunk i+1 executes on Act/Pool/DVE underneath oproj(i)/
passA(i+2), so the PE never waits for q/k eviction. DMA is batched (2 loads +
4 quarter-stores per 512-token chunk); stores issue from the activation engine's
queue right after their evictions.
"""
import sys
sys.path.insert(0, '/opt/trn_rl_repo')

import math
import numpy as np
from contextlib import ExitStack

import concourse.bass as bass
from concourse import bacc
import concourse.mybir as mybir
import concourse.tile as tile
from concourse import bass_utils
from concourse.masks import make_identity

# ---- problem constants (hardcoded per contest contract) ----
B, S, H, HEADS, D = 2, 2048, 2048, 16, 128
NT = B * S                    # 4096 tokens
NCORES = 8
HPC = HEADS // NCORES         # 2 heads per core
OC = HPC * D                  # 256 output dims per core
P = 128
CH = 512                      # token chunk
NCH = NT // CH                # 8 chunks
KT = H // P                   # 16 d-tiles
CPB = S // CH                 # 4 chunks per batch
HD = D // 2
SCALE = 1.0 / math.sqrt(D)
RMS_EPS = 1e-6
ROPE_BASE = 10000.0
MASK_VAL = -10000.0

F32 = mybir.dt.float32
BF16 = mybir.dt.bfloat16
EXP = mybir.ActivationFunctionType.Exp
LN = mybir.ActivationFunctionType.Ln
COPY = mybir.ActivationFunctionType.Copy


def build_module():
    nc = bacc.Bacc("TRN2", target_bir_lowering=False, debug=False, num_devices=NCORES)

    xT = nc.dram_tensor("xT", [H, NT], BF16, kind="ExternalInput").ap()
    wqT = nc.dram_tensor("wqT", [H, OC], BF16, kind="ExternalInput").ap()
    wkT = nc.dram_tensor("wkT", [H, OC], BF16, kind="ExternalInput").ap()
    wvT = nc.dram_tensor("wvT", [H, OC], BF16, kind="ExternalInput").ap()
    woT = nc.dram_tensor("woT", [OC, H], BF16, kind="ExternalInput").ap()
    trigT = nc.dram_tensor("trigT", [2, D, NT], BF16, kind="ExternalInput").ap()
    maskT = nc.dram_tensor("maskT", [P, P], BF16, kind="ExternalInput").ap()
    onesT = nc.dram_tensor("onesT", [P, 1], BF16, kind="ExternalInput").ap()
    identT = nc.dram_tensor("identT", [P, P], BF16, kind="ExternalInput").ap()
    out_p = nc.dram_tensor("out_p", [NT, H], BF16, kind="ExternalOutput").ap()

    xTr = xT.rearrange("(t p) n -> p t n", p=P)
    trig_r = trigT.rearrange("s d n -> d s n")
    out_pr = out_p.rearrange("(c j p) o -> c p j o", j=CPB, p=P)

    with tile.TileContext(nc) as tc, ExitStack() as ctx:
        const = ctx.enter_context(tc.tile_pool(name="const", bufs=1))
        wpool = ctx.enter_context(tc.tile_pool(name="wpool", bufs=1))
        kvpool = ctx.enter_context(tc.tile_pool(name="kvpool", bufs=1))
        xt_pool = ctx.enter_context(tc.tile_pool(name="xtp", bufs=2))
        xq_pool = ctx.enter_context(tc.tile_pool(name="xqp", bufs=16))
        trig_pool = ctx.enter_context(tc.tile_pool(name="trigp", bufs=2))
        trigs_pool = ctx.enter_context(tc.tile_pool(name="trigsp", bufs=2))
        sc_pool = ctx.enter_context(tc.tile_pool(name="scp", bufs=2))
        bc_pool = ctx.enter_context(tc.tile_pool(name="bcp", bufs=2))
        rope_t = ctx.enter_context(tc.tile_pool(name="ropet", bufs=2))
        qk_pool = ctx.enter_context(tc.tile_pool(name="qkev", bufs=8))
        q_pool = ctx.enter_context(tc.tile_pool(name="qp", bufs=4))
        ex_pool = ctx.enter_context(tc.tile_pool(name="exp", bufs=4))
        ctx_pool = ctx.enter_context(tc.tile_pool(name="ctxp", bufs=2))
        o_pool = ctx.enter_context(tc.tile_pool(name="op", bufs=3))
        ps = ctx.enter_context(tc.tile_pool(name="ps", bufs=8, space="PSUM"))

        # ---- small constants (cheap DMAs first) ----
        ones_sb = const.tile([P, 1], BF16)
        nc.sync.dma_start(out=ones_sb, in_=onesT)
        mask_sb = const.tile([P, P], BF16)
        nc.sync.dma_start(out=mask_sb, in_=maskT)
        identB = const.tile([P, P], BF16)
        nc.sync.dma_start(out=identB, in_=identT)
        ident = const.tile([P, P], F32)
        make_identity(nc, ident)
        eps_sb = const.tile([P, 1], F32)
        nc.vector.memset(eps_sb, RMS_EPS)

        wq_sb = wpool.tile([P, KT, OC], BF16)
        wk_sb = wpool.tile([P, KT, OC], BF16)
        wv_sb = wpool.tile([P, KT, OC], BF16)
        wo_sb = wpool.tile([P, HPC, H], BF16)

        # per-chunk K/V caches, resident for the whole kernel
        k_chunks = [kvpool.tile([P, HPC, CH], BF16, name=f"kc{i}") for i in range(NCH)]
        v_chunks = [kvpool.tile([P, CPB, OC], BF16, name=f"vc{i}") for i in range(NCH)]

        cs = [dict() for _ in range(NCH)]   # per-chunk pipeline state

        def emit_loads(i):
            st = cs[i]
            st["xt"] = xt_pool.tile([P, KT, CH], BF16, tag="xta", name=f"xt{i}")
            nc.sync.dma_start(out=st["xt"], in_=xTr[:, :, i * CH:(i + 1) * CH])
            st["trig"] = trig_pool.tile([P, 2, CH], BF16, tag="trig", name=f"tg{i}")
            nc.sync.dma_start(out=st["trig"], in_=trig_r[:, :, i * CH:(i + 1) * CH])

        def emit_passA(i):
            st = cs[i]
            xt_all = st["xt"]
            qp = [ps.tile([P, CH], F32, tag="ps", name=f"qp{i}_{h}") for h in range(HPC)]
            kp = [ps.tile([P, CH], F32, tag="ps", name=f"kp{i}_{h}") for h in range(HPC)]
            vp = [ps.tile([P, 2, OC], F32, tag="ps", name=f"vp{i}_{g}") for g in range(2)]
            stats = ps.tile([P, CPB], F32, tag="ps", name=f"ss{i}")
            xqs = []
            for dt in range(KT):
                xq = xq_pool.tile([P, CH], BF16, tag="xq", name=f"xq{i}_{dt}")
                nc.vector.tensor_mul(xq, xt_all[:, dt, :], xt_all[:, dt, :])
                xqs.append(xq)
            for dt in range(KT):
                st_, sp_ = (dt == 0), (dt == KT - 1)
                for h in range(HPC):
                    nc.tensor.matmul(qp[h], wq_sb[:, dt, h * P:(h + 1) * P],
                                     xt_all[:, dt, :], start=st_, stop=sp_)
            for dt in range(KT):
                st_, sp_ = (dt == 0), (dt == KT - 1)
                for h in range(HPC):
                    nc.tensor.matmul(kp[h], wk_sb[:, dt, h * P:(h + 1) * P],
                                     xt_all[:, dt, :], start=st_, stop=sp_)
            for dt in range(KT):
                st_, sp_ = (dt == 0), (dt == KT - 1)
                for j in range(CPB):
                    nc.tensor.matmul(vp[j // 2][:, j % 2, :],
                                     xt_all[:, dt, j * P:(j + 1) * P],
                                     wv_sb[:, dt, :], start=st_, stop=sp_,
                                     skip_group_check=True)
            for dt in range(KT):
                for j in range(CPB):
                    nc.tensor.matmul(stats[:, j:j + 1], xqs[dt][:, j * P:(j + 1) * P],
                                     ones_sb, start=(dt == 0), stop=(dt == KT - 1),
                                     skip_group_check=True)
            st.update(qe=[qp[0], kp[0], qp[1], kp[1]], vp=vp, stats=stats)

        def emit_rms_vevict(i):
            st = cs[i]
            lnv = sc_pool.tile([P, CPB], F32, tag="ln", name=f"ln{i}")
            nc.scalar.activation(lnv, st["stats"], LN, bias=eps_sb, scale=1.0 / H)
            s_col = sc_pool.tile([P, CPB], F32, tag="sc", name=f"scol{i}")
            nc.scalar.activation(s_col, lnv, EXP, scale=-0.5)
            for j in range(CPB):
                nc.scalar.activation(v_chunks[i][:, j, :], st["vp"][j // 2][:, j % 2, :],
                                     COPY, scale=s_col[:, j:j + 1])
            st["s_col"] = s_col

        def emit_s_transpose(i):
            st = cs[i]
            st["s_rowT_p"] = ps.tile([CPB, P], F32, tag="ps", name=f"srt{i}")
            nc.tensor.transpose(st["s_rowT_p"], st["s_col"], ident)

        def emit_trig_rope(i):
            st = cs[i]
            s_rowT_p = st["s_rowT_p"]
            s_row = sc_pool.tile([1, CH], F32, tag="sr", name=f"srow{i}")
            for j in range(CPB):
                nc.vector.tensor_copy(s_row[0:1, j * P:(j + 1) * P], s_rowT_p[j:j + 1, :])
            s_bc = bc_pool.tile([P, CH], F32, tag="sbc", name=f"sbc{i}")
            nc.gpsimd.partition_broadcast(s_bc, s_row)
            cosS = trigs_pool.tile([P, CH], F32, tag="cosS", name=f"cs{i}")
            nc.vector.tensor_mul(cosS, st["trig"][:, 0, :], s_bc)
            sinS = trigs_pool.tile([P, CH], F32, tag="sinS", name=f"sn{i}")
            nc.vector.tensor_mul(sinS, st["trig"][:, 1, :], s_bc)
            q_sb = []
            for h in range(HPC):
                for (psum_t, dst) in ((st["qe"][2 * h], None), (st["qe"][2 * h + 1], k_chunks[i][:, h, :])):
                    t1 = rope_t.tile([P, CH], F32, tag="t1", name=f"t1_{i}")
                    nc.vector.tensor_mul(t1, psum_t, cosS)
                    t2 = rope_t.tile([P, CH], F32, tag="t2", name=f"t2_{i}")
                    nc.vector.tensor_mul(t2[0:HD, :], psum_t[HD:P, :], sinS[0:HD, :])
                    nc.vector.tensor_mul(t2[HD:P, :], psum_t[0:HD, :], sinS[HD:P, :])
                    if dst is None:
                        dst = q_pool.tile([P, CH], BF16, tag="q", name=f"q{i}_{h}")
                        q_sb.append(dst)
                    nc.vector.tensor_add(dst, t1, t2)
            st["q_sb"] = q_sb

        def emit_attn(i):
            st = cs[i]
            b, li = i // CPB, i % CPB
            nkt = CPB * (li + 1)
            ctx_sb = ctx_pool.tile([P, HPC, CH], BF16, tag="ctx", name=f"ctx{i}")
            den = ps.tile([P, HPC * CPB], F32, tag="ps", name=f"dn{i}")
            ctxs, rec_cols = [], []
            for h in range(HPC):
                ctxp = ps.tile([P, CH], F32, tag="ps", name=f"cx{i}_{h}")
                ctxs.append(ctxp)
                pend = []

                def flush_one(h=h, ctxp=ctxp):
                    kt0, qa0, ex0, ck0, j0 = pend.pop(0)
                    nc.tensor.matmul(ctxp[:, qa0:], v_chunks[ck0][:, j0, h * P:(h + 1) * P],
                                     ex0[:, qa0:], start=(kt0 == 0), stop=(kt0 == nkt - 1),
                                     skip_group_check=True)
                    kk0 = kt0 - CPB * li
                    for j2 in range(CPB):
                        if kk0 <= j2:
                            nc.tensor.matmul(den[:, h * CPB + j2:h * CPB + j2 + 1],
                                             ex0[:, j2 * P:(j2 + 1) * P], ones_sb,
                                             start=(kt0 == 0), stop=(kt0 == CPB * li + j2),
                                             skip_group_check=True)

                for kt in range(nkt):
                    ck = b * CPB + kt // CPB
                    j = kt % CPB
                    kk = kt - CPB * li
                    qa = kk * P if kk > 0 else 0
                    sp_ = ps.tile([P, CH], F32, tag="ps", name=f"s{i}_{h}_{kt}")
                    diag = kk >= 0
                    nc.tensor.matmul(sp_[:, qa:], k_chunks[ck][:, h, j * P:(j + 1) * P],
                                     st["q_sb"][h][:, qa:], start=True, stop=not diag,
                                     skip_group_check=True)
                    if diag:
                        nc.tensor.matmul(sp_[:, kk * P:(kk + 1) * P], mask_sb, identB,
                                         start=False, stop=True, skip_group_check=True)
                    ex = ex_pool.tile([P, CH], BF16, tag="ex", name=f"ex{i}_{h}_{kt}")
                    nc.scalar.activation(ex[:, qa:], sp_[:, qa:], EXP, scale=SCALE)
                    pend.append((kt, qa, ex, ck, j))
                    if len(pend) > 2:
                        flush_one()
                while pend:
                    flush_one()
                rec_col = sc_pool.tile([P, CPB], F32, tag="rc", name=f"rc{i}_{h}")
                nc.vector.reciprocal(rec_col, den[:, h * CPB:(h + 1) * CPB])
                rec_cols.append(rec_col)

            for h in range(HPC):
                recT_p = ps.tile([CPB, P], F32, tag="ps", name=f"rt{i}_{h}")
                nc.tensor.transpose(recT_p, rec_cols[h], ident)
                rec_row = sc_pool.tile([1, CH], F32, tag="rt", name=f"rct{i}_{h}")
                for j in range(CPB):
                    nc.vector.tensor_copy(rec_row[0:1, j * P:(j + 1) * P], recT_p[j:j + 1, :])
                rbc = bc_pool.tile([P, CH], F32, tag="rbc", name=f"rbc{i}_{h}")
                nc.gpsimd.partition_broadcast(rbc, rec_row)
                nc.vector.tensor_mul(ctx_sb[:, h, :], ctxs[h], rbc)
            st["ctx_sb"] = ctx_sb

        def emit_oproj(i, inject=None):
            csb = cs[i]["ctx_sb"]
            for j in range(CPB):
                o_sb = o_pool.tile([P, H], BF16, tag="osb", name=f"ob{i}_{j}")
                for oc in range(H // CH):
                    op_ = ps.tile([P, CH], F32, tag="ps", name=f"o{i}_{j}_{oc}")
                    for h in range(HPC):
                        nc.tensor.matmul(op_, csb[:, h, j * P:(j + 1) * P],
                                         wo_sb[:, h, oc * CH:(oc + 1) * CH],
                                         start=(h == 0), stop=(h == HPC - 1))
                    if (j + oc) % 2 == 0:
                        nc.scalar.copy(o_sb[:, oc * CH:(oc + 1) * CH], op_)
                    else:
                        nc.vector.tensor_copy(o_sb[:, oc * CH:(oc + 1) * CH], op_)
                nc.scalar.dma_start(out=out_pr[i][:, j, :], in_=o_sb)
                if j == 0 and inject is not None:
                    inject()

        # ---- prologue: overlap weight loads with the first chunk's input ----
        nc.sync.dma_start(out=wq_sb, in_=wqT.rearrange("(t p) o -> p t o", p=P))
        emit_loads(0)
        nc.sync.dma_start(out=wk_sb, in_=wkT.rearrange("(t p) o -> p t o", p=P))
        emit_loads(1)
        nc.sync.dma_start(out=wv_sb, in_=wvT.rearrange("(t p) o -> p t o", p=P))
        nc.sync.dma_start(out=wo_sb, in_=woT.rearrange("(t p) o -> p t o", p=P))

        emit_passA(0)
        emit_rms_vevict(0)
        emit_s_transpose(0)
        emit_trig_rope(0)
        for i in range(NCH):
            if i + 1 < NCH:
                emit_passA(i + 1)
                emit_rms_vevict(i + 1)
            if i >= 1:
                emit_oproj(i - 1,
                           inject=(lambda k=i + 1: emit_s_transpose(k)) if i + 1 < NCH else None)
            elif i + 1 < NCH:
                emit_s_transpose(i + 1)
            if i + 1 < NCH:
                emit_trig_rope(i + 1)
                if i + 2 < NCH:
                    emit_loads(i + 2)
            emit_attn(i)
        emit_oproj(NCH - 1)

    nc.compile()
    return nc


def prep_inputs(x, norm_w, wq, wk, wv, wo, position_ids):
    """Host-side sharding/layout prep. Returns per-core input maps."""
    import ml_dtypes
    bf16 = ml_dtypes.bfloat16
    x = np.asarray(x, dtype=np.float32)
    norm_w = np.asarray(norm_w, dtype=np.float32)
    wq = np.asarray(wq, dtype=np.float32)
    wk = np.asarray(wk, dtype=np.float32)
    wv = np.asarray(wv, dtype=np.float32)
    wo = np.asarray(wo, dtype=np.float32)
    pos = np.asarray(position_ids)

    xT = np.ascontiguousarray(x.reshape(NT, H).T).astype(bf16)

    # RoPE tables from position_ids, sign-folded sin
    inv_freq = 1.0 / (ROPE_BASE ** (np.arange(0, D, 2, dtype=np.float32) / D))
    t = pos.reshape(NT).astype(np.float32)
    freqs = np.einsum("n,f->nf", t, inv_freq)
    emb = np.concatenate([freqs, freqs], axis=1)          # [NT, D]
    cos = np.cos(emb).astype(np.float32)
    sin = np.sin(emb).astype(np.float32)
    sinF = sin.copy()
    sinF[:, :HD] *= -1.0
    trigT = np.stack([np.ascontiguousarray(cos.T),
                      np.ascontiguousarray(sinF.T)]).astype(bf16)   # [2, D, NT]

    # transposed triangle mask, added to scores via maskT.T @ I on the PE:
    # MT[qq, kp] = -1e4 where qq < kp (strictly-upper triangle)
    rr = np.arange(P)[:, None]
    cc = np.arange(P)[None, :]
    maskT = np.where(rr < cc, MASK_VAL, 0.0).astype(bf16)

    onesT = np.ones((P, 1), dtype=bf16)
    identT = np.eye(P, dtype=np.float32).astype(bf16)

    wq_f = wq * norm_w[None, :]
    wk_f = wk * norm_w[None, :]
    wv_f = wv * norm_w[None, :]

    in_maps = []
    for c in range(NCORES):
        sl = slice(c * OC, (c + 1) * OC)
        in_maps.append({
            "xT": xT,
            "wqT": np.ascontiguousarray(wq_f[sl].T).astype(bf16),
            "wkT": np.ascontiguousarray(wk_f[sl].T).astype(bf16),
            "wvT": np.ascontiguousarray(wv_f[sl].T).astype(bf16),
            "woT": np.ascontiguousarray(wo[:, sl].T).astype(bf16),
            "trigT": trigT,
            "maskT": maskT,
            "onesT": onesT,
            "identT": identT,
        })
    return in_maps


_NC_CACHE = None


def _get_module():
    global _NC_CACHE
    if _NC_CACHE is None:
        _NC_CACHE = build_module()
    return _NC_CACHE


def kernel(x, norm_w, wq, wk, wv, wo, position_ids):
    nc = _get_module()
    in_maps = prep_inputs(x, norm_w, wq, wk, wv, wo, position_ids)
    res = bass_utils.run_bass_kernel_spmd(nc, in_maps, core_ids=list(range(NCORES)))
    acc = np.zeros((NT, H), dtype=np.float32)
    for c in range(NCORES):
        acc += res.results[c]["out_p"].astype(np.float32)
    return acc.reshape(B, S, H)
